# revision 4
# baseline (speedup 1.0000x reference)
"""Graph attention head (GAT-style) on 8 Trainium2 NeuronCores.

Math (equivalent to the dense reference):
  feats = X @ W1 + b1
  per edge (s,d): score = leaky_relu(p[s] + q[d]), p = feats @ Wa_top, q = feats @ Wa_bot
  alpha = segment_softmax(exp(score), by s);  out[s] = sum_d alpha * feats[d]

Device scheme per core (SPMD, same program, different inputs):
  - Host relabels nodes by descending out-degree, pads to 80 tiles x 128 rows.
    Tile t -> core t%8 slot t//8; each core's own 10 tiles come FIRST in its
    private row order, so the device program is core-agnostic.
  - Phase 1: feats for all 80 tiles via PE ([XT k-tiles] @ [W1|wv_q|wv_p]),
    write [feats|q] rows (fp16) to a DRAM staging table F_aug; keep p columns
    of the 10 own tiles in SBUF.
  - Phase 2 per own tile j: dma_gather F_aug rows by dst for the tile's edge
    slots (one slot = one edge, partition = source node), compute
    ex = exp(leaky(p + q)) batched, denominator by free-dim accumulate,
    aggregate sum_c ex_c * G_c with per-column diag(ex) matmuls into PSUM,
    normalize by 1/denom, DMA out.
Host gathers the 8 per-core [1280,256] outputs and un-permutes rows.
"""
import numpy as np

P = 128
NCORES = 8
N_NODES = 10000
D = 256
NT = 80                    # total row tiles (relabeled+padded rows = 10240)
TPC = NT // NCORES         # tiles per core
NP_ROWS = NT * P           # 10240
PAD_ROW = NP_ROWS          # F_aug row for padding slots (q = -60000 -> ex = 0)
FA_COLS = 384              # F_aug row: [feats(256) | q | unused...], 768B (mult of 256B)
Q_COL = 256
PAD_Q = -60000.0
DEN_EPS = 1e-12

_cache = {}


def _plan(src, dst):
    deg = np.bincount(src, minlength=N_NODES)
    order = np.argsort(-deg, kind="stable")
    inv = np.empty(N_NODES, dtype=np.int64)
    inv[order] = np.arange(N_NODES)
    deg_sorted = deg[order]
    starts = np.zeros(N_NODES + 1, dtype=np.int64)
    np.cumsum(deg, out=starts[1:])
    cols = []
    for j in range(TPC):
        base = 8 * j * P
        cols.append(max(int(deg_sorted[base]) if base < N_NODES else 1, 1))
    return dict(deg=deg, order=order, inv=inv, starts=starts, cols=cols)


def _core_prep(plan, X_rel, dstr, core):
    """Per-core inputs: XT (local row order), wrapped idx array, row maps."""
    cols = plan["cols"]
    C = sum(cols)
    own = [8 * j + core for j in range(TPC)]
    rest = [t for t in range(NT) if (t - core) % 8 != 0]
    local_order = np.array(own + rest, dtype=np.int64)
    glob_of_local = (local_order[:, None] * P + np.arange(P)).ravel()
    g2l = np.empty(NP_ROWS, dtype=np.int64)
    g2l[glob_of_local] = np.arange(NP_ROWS)

    XT = np.ascontiguousarray(X_rel[glob_of_local].T.astype(np.float16))

    deg, order, starts = plan["deg"], plan["order"], plan["starts"]
    dst_slots = np.full((P, C), PAD_ROW, dtype=np.int64)
    c0 = 0
    for j in range(TPC):
        gt = 8 * j + core
        for p in range(P):
            r = gt * P + p
            if r >= N_NODES:
                continue
            o = order[r]
            d = deg[o]
            e0 = starts[o]
            dst_slots[p, c0:c0 + d] = g2l[dstr[e0:e0 + d]]
        c0 += cols[j]

    segs = []
    c0 = 0
    for j in range(TPC):
        seg = dst_slots[:, c0:c0 + cols[j]]          # [128, cj]
        arr = seg.T.reshape(-1)                      # slot i = c*128+p
        segs.append(arr.reshape(-1, 16).T)           # [16, 8*cj]
        c0 += cols[j]
    idx16 = np.concatenate(segs, axis=1).astype(np.int16)
    idx = np.tile(idx16, (8, 1))                     # [128, 8*C]
    return XT, idx, glob_of_local


def _build_program(cols):
    from contextlib import ExitStack
    from concourse import bacc, mybir
    import concourse.tile as tile

    f16, f32, i16 = mybir.dt.float16, mybir.dt.float32, mybir.dt.int16
    Alu = mybir.AluOpType
    C = sum(cols)

    nc = bacc.Bacc("TRN2", target_bir_lowering=False, debug=False,
                   num_devices=NCORES, num_swdge_queues=4)
    xt_d = nc.dram_tensor("xt", [256, NP_ROWS], f16, kind="ExternalInput")
    w_d = nc.dram_tensor("wmat", [256, 258], f16, kind="ExternalInput")
    idx_d = nc.dram_tensor("idx", [128, 8 * C], i16, kind="ExternalInput")
    pad_d = nc.dram_tensor("padrow", [1, FA_COLS], f16, kind="ExternalInput")
    id_d = nc.dram_tensor("ident", [128, 128], f16, kind="ExternalInput")
    out_d = nc.dram_tensor("out", [TPC * P, D], f32, kind="ExternalOutput")

    with tile.TileContext(nc) as tc, ExitStack() as ctx:
        const = ctx.enter_context(tc.tile_pool(name="const", bufs=1))
        psum_f = ctx.enter_context(tc.tile_pool(name="psumf", bufs=2, space="PSUM"))
        psum_a = ctx.enter_context(tc.tile_pool(name="psuma", bufs=2, space="PSUM"))
        fpool = ctx.enter_context(tc.tile_pool(name="fa", bufs=3))
        gpool = ctx.enter_context(tc.tile_pool(name="g", bufs=2))
        spool = ctx.enter_context(tc.tile_pool(name="sc", bufs=2))
        dpool = ctx.enter_context(tc.tile_pool(name="sd", bufs=4))
        opool = ctx.enter_context(tc.tile_pool(name="ob", bufs=2))
        drpool = ctx.enter_context(tc.tile_pool(name="dram", bufs=1, space="DRAM"))

        F_aug = drpool.tile([NP_ROWS + 1, FA_COLS], f16)

        xt_sb = const.tile([128, 2, NP_ROWS], f16)
        nc.sync.dma_start(out=xt_sb[:, 0, :], in_=xt_d[0:128, :])
        nc.sync.dma_start(out=xt_sb[:, 1, :], in_=xt_d[128:256, :])
        w_sb = const.tile([128, 2, 258], f16)
        nc.sync.dma_start(out=w_sb[:, 0, :], in_=w_d[0:128, :])
        nc.sync.dma_start(out=w_sb[:, 1, :], in_=w_d[128:256, :])
        idx_sb = const.tile([128, 8 * C], i16)
        nc.sync.dma_start(out=idx_sb[:], in_=idx_d[:])
        pr = const.tile([1, FA_COLS], f16)
        nc.sync.dma_start(out=pr[:], in_=pad_d[:])
        nc.sync.dma_start(out=F_aug[NP_ROWS:NP_ROWS + 1, :], in_=pr[:])

        ident = const.tile([128, 128], f16)
        nc.sync.dma_start(out=ident[:], in_=id_d[:])

        p_sb = const.tile([128, TPC], f32)

        # ---- Phase 1: feats (+q,p) for all 80 tiles -> F_aug in DRAM ----
        for t in range(NT):
            ps = psum_f.tile([128, 258], f32)
            nc.tensor.matmul(out=ps[:], lhsT=xt_sb[:, 0, t * P:(t + 1) * P],
                             rhs=w_sb[:, 0, :], start=True, stop=False)
            nc.tensor.matmul(out=ps[:], lhsT=xt_sb[:, 1, t * P:(t + 1) * P],
                             rhs=w_sb[:, 1, :], start=False, stop=True)
            fa = fpool.tile([128, 257], f16, tag="fa")
            nc.vector.tensor_copy(out=fa[:], in_=ps[:, 0:257])
            nc.sync.dma_start(out=F_aug[t * P:(t + 1) * P, 0:257], in_=fa[:])
            if t < TPC:
                nc.vector.tensor_copy(out=p_sb[:, t:t + 1], in_=ps[:, 257:258])

        # ---- Phase 2: per own tile: gather, softmax, aggregate ----
        gq = [0]
        c0 = 0
        for j in range(TPC):
            cj = cols[j]
            g = gpool.tile([128, cj, FA_COLS], f16, tag="g")
            # HW: one dma_gather call tolerates <=1024 idxs (desc ring);
            # 8 cols = 1024 idxs. Rotate the 4 SWDGE queues for overlap.
            GCHUNK = 8
            for a in range(0, cj, GCHUNK):
                b = min(a + GCHUNK, cj)
                nc.gpsimd.dma_gather(g[:, a:b, :], F_aug[:, :],
                                     idx_sb[:, 8 * (c0 + a): 8 * (c0 + b)],
                                     128 * (b - a), 128 * (b - a), FA_COLS,
                                     queue_num=gq[0] % 4)
                gq[0] += 1
            qv = g[:, :, Q_COL]                       # [128, cj] fp16 strided
            s5 = spool.tile([128, cj], f32, tag="s5")
            nc.vector.tensor_scalar(out=s5[:], in0=qv, scalar1=p_sb[:, j:j + 1],
                                    scalar2=0.2, op0=Alu.add, op1=Alu.mult)
            s1 = spool.tile([128, cj], f32, tag="s1")
            nc.vector.tensor_scalar_add(out=s1[:], in0=qv,
                                        scalar1=p_sb[:, j:j + 1])
            sl = spool.tile([128, cj], f32, tag="sl")
            nc.vector.tensor_tensor(out=sl[:], in0=s1[:], in1=s5[:], op=Alu.max)
            ex = spool.tile([128, cj], f32, tag="ex")
            den = spool.tile([128, 1], f32, tag="den")
            nc.scalar.activation(out=ex[:], in_=sl[:],
                                 func=mybir.ActivationFunctionType.Exp,
                                 accum_out=den[:])
            den2 = spool.tile([128, 1], f32, tag="den2")
            nc.vector.tensor_scalar_add(out=den2[:], in0=den[:], scalar1=DEN_EPS)
            rec = spool.tile([128, 1], f32, tag="rec")
            nc.vector.reciprocal(out=rec[:], in_=den2[:])

            pa = psum_a.tile([128, D], f32)
            for c in range(cj):
                sd = dpool.tile([128, 128], f16, tag="sd")
                nc.vector.tensor_scalar_mul(out=sd[:], in0=ident[:],
                                            scalar1=ex[:, c:c + 1])
                nc.tensor.matmul(out=pa[:], lhsT=sd[:], rhs=g[:, c, 0:D],
                                 start=(c == 0), stop=(c == cj - 1))
            ob = opool.tile([128, D], f32, tag="ob")
            nc.vector.tensor_scalar_mul(out=ob[:], in0=pa[:], scalar1=rec[:])
            nc.sync.dma_start(out=out_d[j * P:(j + 1) * P, :], in_=ob[:])
            c0 += cj

    nc.compile()
    return nc


def _prep_all(node_features, edges, W1, b1, Wa, ba):
    X = np.asarray(node_features, dtype=np.float32)
    edges = np.asarray(edges)
    W1 = np.asarray(W1, dtype=np.float32)
    b1 = np.asarray(b1, dtype=np.float32)
    Wa = np.asarray(Wa, dtype=np.float32)
    ba = np.asarray(ba, dtype=np.float32)
    assert not np.any(b1) and not np.any(ba), \
        "bias path not implemented (reference uses zero biases)"

    src = edges[:, 0].astype(np.int64)
    dst = edges[:, 1].astype(np.int64)
    if not np.all(src[:-1] <= src[1:]):
        o = np.argsort(src, kind="stable")
        src, dst = src[o], dst[o]

    plan = _plan(src, dst)
    order = plan["order"]
    X_rel = np.zeros((NP_ROWS, D), dtype=np.float32)
    X_rel[:N_NODES] = X[order]
    dstr = plan["inv"][dst]                         # relabeled dst per edge

    wv_q = (W1 @ Wa[256:, 0]).astype(np.float32)
    wv_p = (W1 @ Wa[:256, 0]).astype(np.float32)
    wmat = np.concatenate([W1, wv_q[:, None], wv_p[:, None]],
                          axis=1).astype(np.float16)
    padrow = np.zeros((1, FA_COLS), dtype=np.float16)
    padrow[0, Q_COL] = PAD_Q

    in_maps, gols = [], []
    for core in range(NCORES):
        XT, idx, glob_of_local = _core_prep(plan, X_rel, dstr, core)
        in_maps.append({"xt": XT, "wmat": wmat, "idx": idx, "padrow": padrow,
                        "ident": np.eye(128, dtype=np.float16)})
        gols.append(glob_of_local)
    return plan, in_maps, gols


def kernel(node_features, edges, W1, b1, Wa, ba):
    from concourse.bass_utils import run_bass_kernel_spmd

    plan, in_maps, gols = _prep_all(node_features, edges, W1, b1, Wa, ba)
    key = tuple(plan["cols"])
    if key not in _cache:
        _cache[key] = _build_program(plan["cols"])
    nc = _cache[key]

    res = run_bass_kernel_spmd(nc, in_maps, core_ids=list(range(NCORES)))

    order = plan["order"]
    final = np.zeros((N_NODES, D), dtype=np.float32)
    for core in range(NCORES):
        out = res.results[core]["out"]               # [TPC*P, 256]
        glob_own = gols[core][:TPC * P]              # global relabeled rows
        mask = glob_own < N_NODES
        final[order[glob_own[mask]]] = out[mask]
    return final


# revision 6
# speedup vs baseline: 1.2872x; 1.2872x over previous
"""Graph attention head (GAT-style) on 8 Trainium2 NeuronCores.

Math (equivalent to the dense reference):
  feats = X @ W1 + b1
  per edge (s,d): score = leaky_relu(p[s] + q[d]), p = feats @ Wa_top, q = feats @ Wa_bot
  alpha = segment_softmax(exp(score), by s);  out[s] = sum_d alpha * feats[d]

Device scheme per core (SPMD, same program, different inputs):
  - Host relabels nodes by descending out-degree, pads to 80 tiles x 128 rows.
    Tile t -> core t%8 slot t//8; each core's own 10 tiles come FIRST in its
    private row order, so the device program is core-agnostic.
  - Phase 1: feats for all 80 tiles via PE ([XT k-tiles] @ [W1|wv_q|wv_p]),
    write [feats|q] rows (fp16) to a DRAM staging table F_aug; keep p columns
    of the 10 own tiles in SBUF.
  - Phase 2 per own tile j: dma_gather F_aug rows by dst for the tile's edge
    slots (one slot = one edge, partition = source node), compute
    ex = exp(leaky(p + q)) batched, denominator by free-dim accumulate,
    aggregate sum_c ex_c * G_c with per-column diag(ex) matmuls into PSUM,
    normalize by 1/denom, DMA out.
Host gathers the 8 per-core [1280,256] outputs and un-permutes rows.
"""
import numpy as np

P = 128
NCORES = 8
N_NODES = 10000
D = 256
NT = 80                    # total row tiles (relabeled+padded rows = 10240)
TPC = NT // NCORES         # tiles per core
NP_ROWS = NT * P           # 10240
PAD_ROW = NP_ROWS          # F_aug row for padding slots (q = -60000 -> ex = 0)
FA_COLS = 384              # F_aug row: [feats(256) | q | unused...], 768B (mult of 256B)
Q_COL = 256
PAD_Q = -60000.0
DEN_EPS = 1e-12

_cache = {}


def _plan(src, dst):
    deg = np.bincount(src, minlength=N_NODES)
    order = np.argsort(-deg, kind="stable")
    inv = np.empty(N_NODES, dtype=np.int64)
    inv[order] = np.arange(N_NODES)
    deg_sorted = deg[order]
    starts = np.zeros(N_NODES + 1, dtype=np.int64)
    np.cumsum(deg, out=starts[1:])
    cols = []
    for j in range(TPC):
        base = 8 * j * P
        cols.append(max(int(deg_sorted[base]) if base < N_NODES else 1, 1))
    return dict(deg=deg, order=order, inv=inv, starts=starts, cols=cols)


def _core_prep(plan, X_rel, dstr, core):
    """Per-core inputs: XT (local row order), wrapped idx array, row maps."""
    cols = plan["cols"]
    C = sum(cols)
    own = [8 * j + core for j in range(TPC)]
    rest = [t for t in range(NT) if (t - core) % 8 != 0]
    local_order = np.array(own + rest, dtype=np.int64)
    glob_of_local = (local_order[:, None] * P + np.arange(P)).ravel()
    g2l = np.empty(NP_ROWS, dtype=np.int64)
    g2l[glob_of_local] = np.arange(NP_ROWS)

    XT = np.ascontiguousarray(X_rel[glob_of_local].T.astype(np.float16))

    deg, order, starts = plan["deg"], plan["order"], plan["starts"]
    dst_slots = np.full((P, C), PAD_ROW, dtype=np.int64)
    c0 = 0
    for j in range(TPC):
        gt = 8 * j + core
        for p in range(P):
            r = gt * P + p
            if r >= N_NODES:
                continue
            o = order[r]
            d = deg[o]
            e0 = starts[o]
            dst_slots[p, c0:c0 + d] = g2l[dstr[e0:e0 + d]]
        c0 += cols[j]

    segs = []
    c0 = 0
    for j in range(TPC):
        seg = dst_slots[:, c0:c0 + cols[j]]          # [128, cj]
        arr = seg.T.reshape(-1)                      # slot i = c*128+p
        segs.append(arr.reshape(-1, 16).T)           # [16, 8*cj]
        c0 += cols[j]
    idx16 = np.concatenate(segs, axis=1).astype(np.int16)
    idx = np.tile(idx16, (8, 1))                     # [128, 8*C]
    return XT, idx, glob_of_local


def _build_program(cols):
    from contextlib import ExitStack
    from concourse import bacc, mybir
    import concourse.tile as tile

    f16, f32, i16 = mybir.dt.float16, mybir.dt.float32, mybir.dt.int16
    Alu = mybir.AluOpType
    C = sum(cols)

    nc = bacc.Bacc("TRN2", target_bir_lowering=False, debug=False,
                   num_devices=NCORES, num_swdge_queues=4)
    xt_d = nc.dram_tensor("xt", [256, NP_ROWS], f16, kind="ExternalInput")
    w_d = nc.dram_tensor("wmat", [256, 258], f16, kind="ExternalInput")
    idx_d = nc.dram_tensor("idx", [128, 8 * C], i16, kind="ExternalInput")
    pad_d = nc.dram_tensor("padrow", [1, FA_COLS], f16, kind="ExternalInput")
    id_d = nc.dram_tensor("ident", [128, 128], f16, kind="ExternalInput")
    out_d = nc.dram_tensor("out", [TPC * P, D], f32, kind="ExternalOutput")

    with tile.TileContext(nc) as tc, ExitStack() as ctx:
        const = ctx.enter_context(tc.tile_pool(name="const", bufs=1))
        psum_f = ctx.enter_context(tc.tile_pool(name="psumf", bufs=4, space="PSUM"))
        psum_a = ctx.enter_context(tc.tile_pool(name="psuma", bufs=4, space="PSUM"))
        fpool = ctx.enter_context(tc.tile_pool(name="fa", bufs=2))
        gpool = ctx.enter_context(tc.tile_pool(name="g", bufs=2))
        spool = ctx.enter_context(tc.tile_pool(name="sc", bufs=2))
        dpool = ctx.enter_context(tc.tile_pool(name="sd", bufs=8))
        opool = ctx.enter_context(tc.tile_pool(name="ob", bufs=2))
        drpool = ctx.enter_context(tc.tile_pool(name="dram", bufs=1, space="DRAM"))

        F_aug = drpool.tile([NP_ROWS + 1, FA_COLS], f16)

        xt_sb = const.tile([128, 2, NP_ROWS], f16)
        nc.sync.dma_start(out=xt_sb[:, 0, :], in_=xt_d[0:128, :])
        nc.sync.dma_start(out=xt_sb[:, 1, :], in_=xt_d[128:256, :])
        w_sb = const.tile([128, 2, 258], f16)
        nc.sync.dma_start(out=w_sb[:, 0, :], in_=w_d[0:128, :])
        nc.sync.dma_start(out=w_sb[:, 1, :], in_=w_d[128:256, :])
        idx_sb = const.tile([128, 8 * C], i16)
        nc.sync.dma_start(out=idx_sb[:], in_=idx_d[:])
        pr = const.tile([1, FA_COLS], f16)
        nc.sync.dma_start(out=pr[:], in_=pad_d[:])
        nc.sync.dma_start(out=F_aug[NP_ROWS:NP_ROWS + 1, :], in_=pr[:])

        ident = const.tile([128, 128], f16)
        nc.sync.dma_start(out=ident[:], in_=id_d[:])

        p_sb = const.tile([128, TPC], f32)

        # ---- Phase 1: feats (+q,p) for all 80 tiles -> F_aug in DRAM ----
        FB = 8                     # feats tiles per F_aug write DMA
        fa = None
        for t in range(NT):
            ps = psum_f.tile([128, 258], f32)
            nc.tensor.matmul(out=ps[:], lhsT=xt_sb[:, 0, t * P:(t + 1) * P],
                             rhs=w_sb[:, 0, :], start=True, stop=False)
            nc.tensor.matmul(out=ps[:], lhsT=xt_sb[:, 1, t * P:(t + 1) * P],
                             rhs=w_sb[:, 1, :], start=False, stop=True)
            if t % FB == 0:
                fa = fpool.tile([128, FB, 257], f16, tag="fa")
            # split PSUM->SBUF copies between DVE and ACT (same act table as Exp)
            if t % 2 == 0:
                nc.vector.tensor_copy(out=fa[:, t % FB, :], in_=ps[:, 0:257])
            else:
                nc.scalar.copy(out=fa[:, t % FB, :], in_=ps[:, 0:257])
            if t % FB == FB - 1:
                dst = F_aug[(t - FB + 1) * P:(t + 1) * P, 0:257]
                nc.sync.dma_start(
                    out=dst.rearrange("(k p) c -> p k c", p=P), in_=fa[:])
            if t < TPC:
                nc.vector.tensor_copy(out=p_sb[:, t:t + 1], in_=ps[:, 257:258])

        # ---- Phase 2: per own tile: gather, softmax, aggregate ----
        gq = [0]
        c0 = 0
        for j in range(TPC):
            cj = cols[j]
            g = gpool.tile([128, cj, FA_COLS], f16, tag="g")
            # HW: one dma_gather call tolerates <=1024 idxs (desc ring);
            # 8 cols = 1024 idxs. Rotate the 4 SWDGE queues for overlap.
            GCHUNK = 8
            for a in range(0, cj, GCHUNK):
                b = min(a + GCHUNK, cj)
                nc.gpsimd.dma_gather(g[:, a:b, :], F_aug[:, :],
                                     idx_sb[:, 8 * (c0 + a): 8 * (c0 + b)],
                                     128 * (b - a), 128 * (b - a), FA_COLS,
                                     queue_num=gq[0] % 4)
                gq[0] += 1
            qv = g[:, :, Q_COL]                       # [128, cj] fp16 strided
            s5 = spool.tile([128, cj], f32, tag="s5")
            nc.vector.tensor_scalar(out=s5[:], in0=qv, scalar1=p_sb[:, j:j + 1],
                                    scalar2=0.2, op0=Alu.add, op1=Alu.mult)
            s1 = spool.tile([128, cj], f32, tag="s1")
            nc.vector.tensor_scalar_add(out=s1[:], in0=qv,
                                        scalar1=p_sb[:, j:j + 1])
            sl = spool.tile([128, cj], f32, tag="sl")
            nc.vector.tensor_tensor(out=sl[:], in0=s1[:], in1=s5[:], op=Alu.max)
            ex = spool.tile([128, cj], f32, tag="ex")
            den = spool.tile([128, 1], f32, tag="den")
            nc.scalar.activation(out=ex[:], in_=sl[:],
                                 func=mybir.ActivationFunctionType.Exp,
                                 accum_out=den[:])
            den2 = spool.tile([128, 1], f32, tag="den2")
            nc.vector.tensor_scalar_add(out=den2[:], in0=den[:], scalar1=DEN_EPS)
            rec = spool.tile([128, 1], f32, tag="rec")
            nc.vector.reciprocal(out=rec[:], in_=den2[:])

            pa = psum_a.tile([128, D], f32)
            for c in range(cj):
                sd = dpool.tile([128, 128], f16, tag="sd")
                if c % 5 < 3:
                    nc.vector.tensor_scalar_mul(out=sd[:], in0=ident[:],
                                                scalar1=ex[:, c:c + 1])
                else:
                    nc.scalar.activation(out=sd[:], in_=ident[:],
                                         func=mybir.ActivationFunctionType.Copy,
                                         scale=ex[:, c:c + 1])
                nc.tensor.matmul(out=pa[:], lhsT=sd[:], rhs=g[:, c, 0:D],
                                 start=(c == 0), stop=(c == cj - 1))
            ob = opool.tile([128, D], f32, tag="ob")
            nc.vector.tensor_scalar_mul(out=ob[:], in0=pa[:], scalar1=rec[:])
            nc.sync.dma_start(out=out_d[j * P:(j + 1) * P, :], in_=ob[:])
            c0 += cj

    nc.compile()
    return nc


def _prep_all(node_features, edges, W1, b1, Wa, ba):
    X = np.asarray(node_features, dtype=np.float32)
    edges = np.asarray(edges)
    W1 = np.asarray(W1, dtype=np.float32)
    b1 = np.asarray(b1, dtype=np.float32)
    Wa = np.asarray(Wa, dtype=np.float32)
    ba = np.asarray(ba, dtype=np.float32)
    assert not np.any(b1) and not np.any(ba), \
        "bias path not implemented (reference uses zero biases)"

    src = edges[:, 0].astype(np.int64)
    dst = edges[:, 1].astype(np.int64)
    if not np.all(src[:-1] <= src[1:]):
        o = np.argsort(src, kind="stable")
        src, dst = src[o], dst[o]

    plan = _plan(src, dst)
    order = plan["order"]
    X_rel = np.zeros((NP_ROWS, D), dtype=np.float32)
    X_rel[:N_NODES] = X[order]
    dstr = plan["inv"][dst]                         # relabeled dst per edge

    wv_q = (W1 @ Wa[256:, 0]).astype(np.float32)
    wv_p = (W1 @ Wa[:256, 0]).astype(np.float32)
    wmat = np.concatenate([W1, wv_q[:, None], wv_p[:, None]],
                          axis=1).astype(np.float16)
    padrow = np.zeros((1, FA_COLS), dtype=np.float16)
    padrow[0, Q_COL] = PAD_Q

    in_maps, gols = [], []
    for core in range(NCORES):
        XT, idx, glob_of_local = _core_prep(plan, X_rel, dstr, core)
        in_maps.append({"xt": XT, "wmat": wmat, "idx": idx, "padrow": padrow,
                        "ident": np.eye(128, dtype=np.float16)})
        gols.append(glob_of_local)
    return plan, in_maps, gols


def kernel(node_features, edges, W1, b1, Wa, ba):
    from concourse.bass_utils import run_bass_kernel_spmd

    plan, in_maps, gols = _prep_all(node_features, edges, W1, b1, Wa, ba)
    key = tuple(plan["cols"])
    if key not in _cache:
        _cache[key] = _build_program(plan["cols"])
    nc = _cache[key]

    res = run_bass_kernel_spmd(nc, in_maps, core_ids=list(range(NCORES)))

    order = plan["order"]
    final = np.zeros((N_NODES, D), dtype=np.float32)
    for core in range(NCORES):
        out = res.results[core]["out"]               # [TPC*P, 256]
        glob_own = gols[core][:TPC * P]              # global relabeled rows
        mask = glob_own < N_NODES
        final[order[glob_own[mask]]] = out[mask]
    return final
